# revision 8
# baseline (speedup 1.0000x reference)
"""Trainium2 Bass kernel: per-pixel channel shuffle + 3x3 conv (stride 1, pad 1).

Problem: x [32,256,56,56] f32, w [256,256,3,3] f32 (OIHW), perm [3136,256] i32;
out[b,:,h,w] = conv3x3(xs)[b,:,h,w] where xs[b,:,l] = x[b, perm[l,:], l].

Strategy (8 NeuronCores, data-parallel over batch, 4 batches/core):
  The 4 local batches form one continuous 12544-pixel stream (98 groups of
  128 px).  Per scatter call (k<=7 groups): DMA a [128, k*256] bf16 tile
  (partition = pixel-within-group, free = group-major channels), one big
  GPSIMD local_scatter applies each pixel's inverse channel permutation AND
  rearranges channels ct-major (dst index table built on host, period 49
  groups = 2 images), then one batched DMA-transpose per 128-channel ic-tile
  ([128, k*128] -> [128, k, 128]) lands [channel, pixel-run] data in a
  staging ring.  DVE copies insert 57-wide row padding (56 px + 1 zero) into
  per-batch image planes [128, 58*57] (+1 lead zero col) so the 3x3 conv is
  pure implicit GEMM: per batch 7 row-groups x 2 oc-tiles x 18 matmuls
  (9 taps x 2 ic-tiles) of [128 x 456] accumulate in PSUM; ACT evicts,
  scalar-queue DMA strips the padding on the way to HBM.  PE runs only conv
  matmuls (~192us roofline/core); GPSIMD ~125us; all overlapped.
"""

import sys
import types
import numpy as np

_STATE = {}
LAST_RESULT = None

B, C, H, W = 32, 256, 56, 56
HW = H * W          # 3136
N_CORES = 8
B_LOC = B // N_CORES
GPX = B_LOC * HW    # 12544 pixels per core
NGRP = GPX // 128   # 98 groups of 128 px
PERIOD = 49         # idx table period in groups (2 images: 49*128 = 2*3136)
ROWW = 57           # padded row width (56 + 1 zero col)
REG = 1 + 58 * ROWW + 1   # per-ic-tile plane region (lead zero + 58 rows + slack)
NG = 8 * ROWW       # conv group free size = 456 (8 output rows)

# scatter calls: (start_group, n_groups). Small head calls start the conv early.
CALLS = [(0, 2), (2, 2), (4, 3)] + [(7 + 7 * i, 7) for i in range(13)]
STG_SLOTS = 6


def _install_ntff_shim():
    # antenv.axon_hooks is absent in some images; provide it so trace=True
    # (BASS_TRACE=1) can capture NTFF profiles instead of crashing.
    name = "antenv.axon_hooks"
    if name in sys.modules:
        return
    try:
        import antenv  # noqa: F401

        m = types.ModuleType(name)
        m._hook = None
        m.set_axon_ntff_profile_hook = lambda h: setattr(m, "_hook", h)
        m.get_axon_ntff_profile_hook = lambda: m._hook
        sys.modules[name] = m
        setattr(sys.modules["antenv"], "axon_hooks", m)
        from trn_agent_boot.trn_boot import _ntff_profile_via_ctypes

        hook = _ntff_profile_via_ctypes("/opt/axon/libaxon_pjrt.so")
        if hook is not None:
            m.set_axon_ntff_profile_hook(hook)
    except Exception:
        pass


def _copy_segments():
    """Per call: list of (ct, slot_off, plane_idx, plane_off, kind, n)
    kind 'rows': full rows block (src [128, n*56] -> dst [[57,n],[1,56]])
    kind 'part': partial row of n px."""
    segs = []
    ready = []  # per call: list of (b, g) conv groups newly ready
    done_rows = 0
    issued = set()
    for m, (s, k) in enumerate(CALLS):
        p0, p1 = 128 * s, 128 * (s + k)
        cur = []
        a = p0
        while a < p1:
            r = a // 56
            row_end = 56 * (r + 1)
            bnd = min(p1, row_end)
            b_idx = r // 56
            rl = r % 56
            if a == 56 * r and bnd == row_end:
                # run of full rows: extend while same batch
                r2 = r
                while (
                    56 * (r2 + 1) <= p1
                    and (r2 // 56) == b_idx
                ):
                    r2 += 1
                n = r2 - r
                cur.append((a - p0, b_idx, 1 + (1 + rl) * ROWW, "rows", n))
                a = 56 * r2
            else:
                n = bnd - a
                cur.append((a - p0, b_idx, 1 + (1 + rl) * ROWW + (a - 56 * r), "part", n))
                a = bnd
        segs.append(cur)
        done_rows = p1 // 56
        newly = []
        for b in range(B_LOC):
            for g in range(7):
                if (b, g) in issued:
                    continue
                if 56 * b + min(8 * g + 9, 56) <= done_rows:
                    issued.add((b, g))
                    newly.append((b, g))
        ready.append(newly)
    assert len(issued) == B_LOC * 7
    return segs, ready


def _build_kernel():
    import concourse.bass as bass
    import concourse.mybir as mybir
    from concourse import bacc, tile
    from contextlib import ExitStack

    F32 = mybir.dt.float32
    BF16 = mybir.dt.bfloat16
    I16 = mybir.dt.int16

    nc = bacc.Bacc("TRN2", target_bir_lowering=False, debug=False, num_devices=N_CORES)

    xt = nc.dram_tensor("xt", [GPX, C], BF16, kind="ExternalInput")
    wt = nc.dram_tensor("wt", [36, 128, 128], BF16, kind="ExternalInput")
    idxh = nc.dram_tensor("idxh", [128, 7 * 256], I16, kind="ExternalInput")
    idxs = nc.dram_tensor("idxs", [128, PERIOD * 256], I16, kind="ExternalInput")
    out = nc.dram_tensor("out", [B_LOC, C, HW], F32, kind="ExternalOutput")

    segs, ready = _copy_segments()

    with tile.TileContext(nc) as tc, ExitStack() as ctx:
        const = ctx.enter_context(tc.tile_pool(name="const", bufs=1))

        # Pre-warm the GPSIMD local_scatter library (~6us IRAM load).
        dd = const.tile([16, 256], BF16, name="dd", tag="dd")
        nc.vector.memset(dd[:, :], 0.0)
        didx = const.tile([16, 16], I16, name="didx", tag="didx")
        nc.vector.memset(didx[:, :], -1)
        dout = const.tile([16, 256], BF16, name="dout", tag="dout")
        nc.gpsimd.local_scatter(
            out_ap=dout[:, :], data_ap=dd[:, :], idxs_ap=didx[:, :],
            channels=16, num_elems=256, num_idxs=16,
        )

        idxh_sb = const.tile([128, 7 * 256], I16)
        idxs_sb = const.tile([128, PERIOD * 256], I16)
        wsb = const.tile([128, 36 * 128], BF16)

        # double-buffered padded image planes, 2 ic-tile regions each
        planes = []
        for pi in range(2):
            pl = const.tile([128, 2 * REG], BF16, name=f"pl{pi}", tag=f"pl{pi}")
            nc.vector.memset(pl[:, :], 0.0)
            planes.append(pl)

        # staging rings for transposed [channel, pixel] data, one per ic-tile
        stg = [
            const.tile([128, STG_SLOTS * 896], BF16, name=f"stg{ct}", tag=f"stg{ct}")
            for ct in range(2)
        ]

        nc.sync.dma_start(out=idxh_sb[:, :], in_=idxh[:, :])
        nc.scalar.dma_start(
            out=wsb[:, :],
            in_=bass.AP(wt, 0, [[128, 128], [128 * 128, 36], [1, 128]]),
        )
        nc.scalar.dma_start(
            out=idxs_sb[:, : 25 * 256], in_=idxs[:, : 25 * 256]
        )
        nc.scalar.dma_start(
            out=idxs_sb[:, 25 * 256 :], in_=idxs[:, 25 * 256 :]
        )

        xin_pool = ctx.enter_context(tc.tile_pool(name="xin", bufs=3))
        sout_pool = ctx.enter_context(tc.tile_pool(name="sout", bufs=3))
        ost_pool = ctx.enter_context(tc.tile_pool(name="ost", bufs=4))
        mpsum_pool = ctx.enter_context(tc.tile_pool(name="mpsum", bufs=6, space="PSUM"))

        def conv_group(b, g):
            pl = planes[b % 2]
            for oct in range(2):
                mp = mpsum_pool.tile([128, NG], F32, name="mp", tag="mp")
                for i in range(18):
                    ct, tap = divmod(i, 9)
                    dh, dw = divmod(tap, 3)
                    q0 = ct * REG + (8 * g + dh) * ROWW + dw
                    widx = (ct * 9 + tap) * 2 + oct
                    nc.tensor.matmul(
                        mp[:, :],
                        lhsT=wsb[:, widx * 128 : (widx + 1) * 128],
                        rhs=pl[:, q0 : q0 + NG],
                        start=(i == 0),
                        stop=(i == 17),
                    )
                ost = ost_pool.tile([128, NG], F32, name="ost", tag="ost")
                nc.scalar.copy(ost[:, :], mp[:, :])
                nc.scalar.dma_start(
                    out=out[
                        b, oct * 128 : (oct + 1) * 128, 448 * g : 448 * g + 448
                    ],
                    in_=ost[:, :].rearrange("p (r x) -> p r x", r=8)[:, :, 0:56],
                )

        # xin prefetch 2 calls ahead so transposes (which wait on the
        # scatter) never block the next input load on the sync FIFO
        xins = {}

        def load_xin(m):
            if m >= len(CALLS):
                return
            s, k = CALLS[m]
            kk = k * 256
            xin = xin_pool.tile([128, 7 * 256], BF16, name="xin", tag="xin")
            nc.sync.dma_start(
                out=xin[:, :kk],
                in_=bass.AP(xt, 128 * s * C, [[C, 128], [128 * C, k], [1, C]]),
            )
            xins[m] = xin

        def do_copies(mi):
            slot_i = (mi % STG_SLOTS) * 896
            for src_off, b_idx, pl_off, kind, n in segs[mi]:
                pl = planes[b_idx % 2]
                for ct in range(2):
                    so = slot_i + src_off
                    po = ct * REG + pl_off
                    if kind == "rows":
                        src = stg[ct][:, so : so + 56 * n].rearrange(
                            "p (r x) -> p r x", r=n
                        )
                        dst = pl[:, po : po + n * ROWW].rearrange(
                            "p (r x) -> p r x", r=n
                        )[:, :, 0:56]
                        nc.scalar.copy(dst, src)
                    else:
                        nc.scalar.copy(
                            pl[:, po : po + n], stg[ct][:, so : so + n]
                        )

        load_xin(0)
        load_xin(1)
        for m, (s, k) in enumerate(CALLS):
            kk = k * 256
            load_xin(m + 2)
            xin = xins.pop(m)
            if m < 3:
                iap = idxh_sb[:, s * 256 : (s + k) * 256]
            else:
                sp = (s % PERIOD) * 256
                iap = idxs_sb[:, sp : sp + kk]
            sout = sout_pool.tile([128, 7 * 256], BF16, name="sout", tag="sout")
            nc.gpsimd.local_scatter(
                out_ap=sout[:, :kk],
                data_ap=xin[:, :kk],
                idxs_ap=iap,
                channels=128,
                num_elems=kk,
                num_idxs=kk,
            )
            slot = (m % STG_SLOTS) * 896
            for ct in range(2):
                dst = stg[ct][:, slot : slot + k * 128].rearrange(
                    "p (e l) -> p e l", e=k
                )
                nc.sync.dma_start_transpose(
                    dst, sout[:, ct * k * 128 : (ct + 1) * k * 128]
                )
            # pad-copies on ACT (own SBUF port; DVE contends with GPSIMD).
            # Copies lag one call (their transpose is already done -> the
            # scalar FIFO head never waits on an in-flight scatter) and conv
            # groups lag two calls (so an eviction at the FIFO head only
            # delays copies that PE won't need for another full call).
            if m >= 1:
                do_copies(m - 1)
            if m >= 2:
                for (b, g) in ready[m - 2]:
                    conv_group(b, g)
        do_copies(len(CALLS) - 1)
        for mm in (len(CALLS) - 2, len(CALLS) - 1):
            for (b, g) in ready[mm]:
                conv_group(b, g)

    nc.compile()
    return nc


def _host_prep(x, w, perm):
    import ml_dtypes

    # pixel-major bf16: [B, HW, C]
    xf = np.ascontiguousarray(
        x.reshape(B, C, HW).transpose(0, 2, 1)
    ).astype(ml_dtypes.bfloat16)

    wt = np.empty((36, 128, 128), dtype=ml_dtypes.bfloat16)
    wf = np.asarray(w, dtype=np.float32)
    for ct in range(2):
        for tap in range(9):
            kh, kw = divmod(tap, 3)
            for oct in range(2):
                i = (ct * 9 + tap) * 2 + oct
                wt[i] = wf[
                    oct * 128 : (oct + 1) * 128, ct * 128 : (ct + 1) * 128, kh, kw
                ].T.astype(ml_dtypes.bfloat16)

    iperm = np.empty((HW, C), dtype=np.int16)
    np.put_along_axis(
        iperm, perm.astype(np.int64), np.arange(C, dtype=np.int16)[None, :], axis=1
    )

    # steady idx table [128, PERIOD*256]: group g, partition p -> pixel
    # (128g+p) % HW; dst = ct_major(k=7): (ip>>7)*896 + (g%7)*128 + (ip&127)
    gg = np.arange(PERIOD)
    pp = np.arange(128)
    l = (128 * gg[:, None] + pp[None, :]) % HW          # [49, 128]
    ip = iperm[l].astype(np.int32)                      # [49, 128, 256]
    j = (gg % 7).astype(np.int32)[:, None, None]
    dst = (ip >> 7) * 896 + j * 128 + (ip & 127)
    idxs = np.ascontiguousarray(
        dst.astype(np.int16).transpose(1, 0, 2).reshape(128, PERIOD * 256)
    )

    # head table for the first 7 groups with call sizes (0,2),(2,2),(4,3)
    idxh = np.empty((128, 7 * 256), dtype=np.int16)
    for (s, k) in CALLS[:3]:
        for g in range(s, s + k):
            lg = (128 * g + pp) % HW
            ipg = iperm[lg].astype(np.int32)            # [128, 256]
            d = (ipg >> 7) * (k * 128) + (g - s) * 128 + (ipg & 127)
            idxh[:, g * 256 : (g + 1) * 256] = d.astype(np.int16)

    in_maps = []
    for cidx in range(N_CORES):
        in_maps.append(
            {
                "xt": np.ascontiguousarray(
                    xf[cidx * B_LOC : (cidx + 1) * B_LOC].reshape(GPX, C)
                ),
                "wt": wt,
                "idxh": idxh,
                "idxs": idxs,
            }
        )
    return in_maps


def kernel(x, w, perm):
    global LAST_RESULT
    _install_ntff_shim()
    from concourse.bass_utils import run_bass_kernel_spmd

    x = np.asarray(x, dtype=np.float32)
    w = np.asarray(w, dtype=np.float32)
    perm = np.asarray(perm)

    if "nc" not in _STATE:
        _STATE["nc"] = _build_kernel()
    nc = _STATE["nc"]

    in_maps = _host_prep(x, w, perm)
    res = run_bass_kernel_spmd(nc, in_maps, core_ids=list(range(N_CORES)))
    LAST_RESULT = res
    out = np.concatenate(
        [r["out"].reshape(B_LOC, C, H, W) for r in res.results], axis=0
    )
    return out.astype(np.float32)


# revision 9
# speedup vs baseline: 1.0305x; 1.0305x over previous
"""Trainium2 Bass kernel: per-pixel channel shuffle + 3x3 conv (stride 1, pad 1).

Problem: x [32,256,56,56] f32, w [256,256,3,3] f32 (OIHW), perm [3136,256] i32;
out[b,:,h,w] = conv3x3(xs)[b,:,h,w] where xs[b,:,l] = x[b, perm[l,:], l].

Strategy (8 NeuronCores, data-parallel over batch, 4 batches/core):
  The 4 local batches form one continuous 12544-pixel stream (98 groups of
  128 px).  Per scatter call (k<=7 groups): DMA a [128, k*256] bf16 tile
  (partition = pixel-within-group, free = group-major channels), one big
  GPSIMD local_scatter applies each pixel's inverse channel permutation AND
  rearranges channels ct-major (dst index table built on host, period 49
  groups = 2 images), then one batched DMA-transpose per 128-channel ic-tile
  ([128, k*128] -> [128, k, 128]) lands [channel, pixel-run] data in a
  staging ring.  DVE copies insert 57-wide row padding (56 px + 1 zero) into
  per-batch image planes [128, 58*57] (+1 lead zero col) so the 3x3 conv is
  pure implicit GEMM: per batch 7 row-groups x 2 oc-tiles x 18 matmuls
  (9 taps x 2 ic-tiles) of [128 x 456] accumulate in PSUM; ACT evicts,
  scalar-queue DMA strips the padding on the way to HBM.  PE runs only conv
  matmuls (~192us roofline/core); GPSIMD ~125us; all overlapped.
"""

import sys
import types
import numpy as np

_STATE = {}
LAST_RESULT = None

B, C, H, W = 32, 256, 56, 56
HW = H * W          # 3136
N_CORES = 8
B_LOC = B // N_CORES
GPX = B_LOC * HW    # 12544 pixels per core
NGRP = GPX // 128   # 98 groups of 128 px
PERIOD = 49         # idx table period in groups (2 images: 49*128 = 2*3136)
ROWW = 57           # padded row width (56 + 1 zero col)
REG = 1 + 58 * ROWW + 1   # per-ic-tile plane region (lead zero + 58 rows + slack)
NG = 8 * ROWW       # conv group free size = 456 (8 output rows)

# scatter calls: (start_group, n_groups). Small head calls start the conv early.
CALLS = [(0, 2), (2, 2), (4, 3)] + [(7 + 7 * i, 7) for i in range(13)]
STG_SLOTS = 6


def _install_ntff_shim():
    # antenv.axon_hooks is absent in some images; provide it so trace=True
    # (BASS_TRACE=1) can capture NTFF profiles instead of crashing.
    name = "antenv.axon_hooks"
    if name in sys.modules:
        return
    try:
        import antenv  # noqa: F401

        m = types.ModuleType(name)
        m._hook = None
        m.set_axon_ntff_profile_hook = lambda h: setattr(m, "_hook", h)
        m.get_axon_ntff_profile_hook = lambda: m._hook
        sys.modules[name] = m
        setattr(sys.modules["antenv"], "axon_hooks", m)
        from trn_agent_boot.trn_boot import _ntff_profile_via_ctypes

        hook = _ntff_profile_via_ctypes("/opt/axon/libaxon_pjrt.so")
        if hook is not None:
            m.set_axon_ntff_profile_hook(hook)
    except Exception:
        pass


def _copy_segments():
    """Per call: list of (ct, slot_off, plane_idx, plane_off, kind, n)
    kind 'rows': full rows block (src [128, n*56] -> dst [[57,n],[1,56]])
    kind 'part': partial row of n px."""
    segs = []
    ready = []  # per call: list of (b, g) conv groups newly ready
    done_rows = 0
    issued = set()
    for m, (s, k) in enumerate(CALLS):
        p0, p1 = 128 * s, 128 * (s + k)
        cur = []
        a = p0
        while a < p1:
            r = a // 56
            row_end = 56 * (r + 1)
            bnd = min(p1, row_end)
            b_idx = r // 56
            rl = r % 56
            if a == 56 * r and bnd == row_end:
                # run of full rows: extend while same batch
                r2 = r
                while (
                    56 * (r2 + 1) <= p1
                    and (r2 // 56) == b_idx
                ):
                    r2 += 1
                n = r2 - r
                cur.append((a - p0, b_idx, 1 + (1 + rl) * ROWW, "rows", n))
                a = 56 * r2
            else:
                n = bnd - a
                cur.append((a - p0, b_idx, 1 + (1 + rl) * ROWW + (a - 56 * r), "part", n))
                a = bnd
        segs.append(cur)
        done_rows = p1 // 56
        newly = []
        for b in range(B_LOC):
            for g in range(7):
                if (b, g) in issued:
                    continue
                if 56 * b + min(8 * g + 9, 56) <= done_rows:
                    issued.add((b, g))
                    newly.append((b, g))
        ready.append(newly)
    assert len(issued) == B_LOC * 7
    return segs, ready


def _build_kernel():
    import concourse.bass as bass
    import concourse.mybir as mybir
    from concourse import bacc, tile
    from contextlib import ExitStack

    F32 = mybir.dt.float32
    BF16 = mybir.dt.bfloat16
    I16 = mybir.dt.int16

    nc = bacc.Bacc("TRN2", target_bir_lowering=False, debug=False, num_devices=N_CORES)

    xt = nc.dram_tensor("xt", [GPX, C], BF16, kind="ExternalInput")
    wt = nc.dram_tensor("wt", [36, 128, 128], BF16, kind="ExternalInput")
    idxh = nc.dram_tensor("idxh", [128, 7 * 256], I16, kind="ExternalInput")
    idxs = nc.dram_tensor("idxs", [128, PERIOD * 256], I16, kind="ExternalInput")
    out = nc.dram_tensor("out", [B_LOC, C, HW], F32, kind="ExternalOutput")

    segs, ready = _copy_segments()

    with tile.TileContext(nc) as tc, ExitStack() as ctx:
        const = ctx.enter_context(tc.tile_pool(name="const", bufs=1))

        # Pre-warm the GPSIMD local_scatter library (~6us IRAM load).
        dd = const.tile([16, 256], BF16, name="dd", tag="dd")
        nc.vector.memset(dd[:, :], 0.0)
        didx = const.tile([16, 16], I16, name="didx", tag="didx")
        nc.vector.memset(didx[:, :], -1)
        dout = const.tile([16, 256], BF16, name="dout", tag="dout")
        nc.gpsimd.local_scatter(
            out_ap=dout[:, :], data_ap=dd[:, :], idxs_ap=didx[:, :],
            channels=16, num_elems=256, num_idxs=16,
        )

        idxh_sb = const.tile([128, 7 * 256], I16)
        idxs_sb = const.tile([128, PERIOD * 256], I16)
        wsb = const.tile([128, 36 * 128], BF16)

        # double-buffered padded image planes, 2 ic-tile regions each
        planes = []
        for pi in range(2):
            pl = const.tile([128, 2 * REG], BF16, name=f"pl{pi}", tag=f"pl{pi}")
            nc.vector.memset(pl[:, :], 0.0)
            planes.append(pl)

        # staging rings for transposed [channel, pixel] data, one per ic-tile
        stg = [
            const.tile([128, STG_SLOTS * 896], BF16, name=f"stg{ct}", tag=f"stg{ct}")
            for ct in range(2)
        ]

        nc.sync.dma_start(out=idxh_sb[:, :], in_=idxh[:, :])
        nc.scalar.dma_start(
            out=wsb[:, :],
            in_=bass.AP(wt, 0, [[128, 128], [128 * 128, 36], [1, 128]]),
        )
        nc.scalar.dma_start(
            out=idxs_sb[:, : 25 * 256], in_=idxs[:, : 25 * 256]
        )
        nc.scalar.dma_start(
            out=idxs_sb[:, 25 * 256 :], in_=idxs[:, 25 * 256 :]
        )

        xin_pool = ctx.enter_context(tc.tile_pool(name="xin", bufs=3))
        sout_pool = ctx.enter_context(tc.tile_pool(name="sout", bufs=3))
        ost_pool = ctx.enter_context(tc.tile_pool(name="ost", bufs=4))
        mpsum_pool = ctx.enter_context(tc.tile_pool(name="mpsum", bufs=6, space="PSUM"))

        def conv_group(b, g):
            pl = planes[b % 2]
            for oct in range(2):
                mp = mpsum_pool.tile([128, NG], F32, name="mp", tag="mp")
                for i in range(18):
                    ct, tap = divmod(i, 9)
                    dh, dw = divmod(tap, 3)
                    q0 = ct * REG + (8 * g + dh) * ROWW + dw
                    widx = (ct * 9 + tap) * 2 + oct
                    nc.tensor.matmul(
                        mp[:, :],
                        lhsT=wsb[:, widx * 128 : (widx + 1) * 128],
                        rhs=pl[:, q0 : q0 + NG],
                        start=(i == 0),
                        stop=(i == 17),
                    )
                ost = ost_pool.tile([128, NG], F32, name="ost", tag="ost")
                nc.scalar.copy(ost[:, :], mp[:, :])
                # out-DMA on sync: ALL DMAs are framework-serialized around
                # DMA-transposes, so they share the transpose queue; scalar
                # stays a pure compute queue (evictions+copies never blocked)
                nc.sync.dma_start(
                    out=out[
                        b, oct * 128 : (oct + 1) * 128, 448 * g : 448 * g + 448
                    ],
                    in_=ost[:, :].rearrange("p (r x) -> p r x", r=8)[:, :, 0:56],
                )

        # xin prefetch 2 calls ahead so transposes (which wait on the
        # scatter) never block the next input load on the sync FIFO
        xins = {}

        def load_xin(m):
            if m >= len(CALLS):
                return
            s, k = CALLS[m]
            kk = k * 256
            xin = xin_pool.tile([128, 7 * 256], BF16, name="xin", tag="xin")
            nc.sync.dma_start(
                out=xin[:, :kk],
                in_=bass.AP(xt, 128 * s * C, [[C, 128], [128 * C, k], [1, C]]),
            )
            xins[m] = xin

        def do_copies(mi):
            slot_i = (mi % STG_SLOTS) * 896
            for src_off, b_idx, pl_off, kind, n in segs[mi]:
                pl = planes[b_idx % 2]
                for ct in range(2):
                    so = slot_i + src_off
                    po = ct * REG + pl_off
                    if kind == "rows":
                        src = stg[ct][:, so : so + 56 * n].rearrange(
                            "p (r x) -> p r x", r=n
                        )
                        dst = pl[:, po : po + n * ROWW].rearrange(
                            "p (r x) -> p r x", r=n
                        )[:, :, 0:56]
                        nc.scalar.copy(dst, src)
                    else:
                        nc.scalar.copy(
                            pl[:, po : po + n], stg[ct][:, so : so + n]
                        )

        load_xin(0)
        load_xin(1)
        for m, (s, k) in enumerate(CALLS):
            kk = k * 256
            load_xin(m + 2)
            xin = xins.pop(m)
            if m < 3:
                iap = idxh_sb[:, s * 256 : (s + k) * 256]
            else:
                sp = (s % PERIOD) * 256
                iap = idxs_sb[:, sp : sp + kk]
            sout = sout_pool.tile([128, 7 * 256], BF16, name="sout", tag="sout")
            nc.gpsimd.local_scatter(
                out_ap=sout[:, :kk],
                data_ap=xin[:, :kk],
                idxs_ap=iap,
                channels=128,
                num_elems=kk,
                num_idxs=kk,
            )
            slot = (m % STG_SLOTS) * 896
            for ct in range(2):
                dst = stg[ct][:, slot : slot + k * 128].rearrange(
                    "p (e l) -> p e l", e=k
                )
                nc.sync.dma_start_transpose(
                    dst, sout[:, ct * k * 128 : (ct + 1) * k * 128]
                )
            # pad-copies on ACT (own SBUF port; DVE contends with GPSIMD).
            # Copies lag one call (their transpose is already done -> the
            # scalar FIFO head never waits on an in-flight scatter) and conv
            # groups lag two calls (so an eviction at the FIFO head only
            # delays copies that PE won't need for another full call).
            if m >= 1:
                do_copies(m - 1)
            if m >= 2:
                for (b, g) in ready[m - 2]:
                    conv_group(b, g)
        do_copies(len(CALLS) - 1)
        for mm in (len(CALLS) - 2, len(CALLS) - 1):
            for (b, g) in ready[mm]:
                conv_group(b, g)

    nc.compile()
    return nc


def _host_prep(x, w, perm):
    import ml_dtypes

    # pixel-major bf16: [B, HW, C]
    xf = np.ascontiguousarray(
        x.reshape(B, C, HW).transpose(0, 2, 1)
    ).astype(ml_dtypes.bfloat16)

    wt = np.empty((36, 128, 128), dtype=ml_dtypes.bfloat16)
    wf = np.asarray(w, dtype=np.float32)
    for ct in range(2):
        for tap in range(9):
            kh, kw = divmod(tap, 3)
            for oct in range(2):
                i = (ct * 9 + tap) * 2 + oct
                wt[i] = wf[
                    oct * 128 : (oct + 1) * 128, ct * 128 : (ct + 1) * 128, kh, kw
                ].T.astype(ml_dtypes.bfloat16)

    iperm = np.empty((HW, C), dtype=np.int16)
    np.put_along_axis(
        iperm, perm.astype(np.int64), np.arange(C, dtype=np.int16)[None, :], axis=1
    )

    # steady idx table [128, PERIOD*256]: group g, partition p -> pixel
    # (128g+p) % HW; dst = ct_major(k=7): (ip>>7)*896 + (g%7)*128 + (ip&127)
    gg = np.arange(PERIOD)
    pp = np.arange(128)
    l = (128 * gg[:, None] + pp[None, :]) % HW          # [49, 128]
    ip = iperm[l].astype(np.int32)                      # [49, 128, 256]
    j = (gg % 7).astype(np.int32)[:, None, None]
    dst = (ip >> 7) * 896 + j * 128 + (ip & 127)
    idxs = np.ascontiguousarray(
        dst.astype(np.int16).transpose(1, 0, 2).reshape(128, PERIOD * 256)
    )

    # head table for the first 7 groups with call sizes (0,2),(2,2),(4,3)
    idxh = np.empty((128, 7 * 256), dtype=np.int16)
    for (s, k) in CALLS[:3]:
        for g in range(s, s + k):
            lg = (128 * g + pp) % HW
            ipg = iperm[lg].astype(np.int32)            # [128, 256]
            d = (ipg >> 7) * (k * 128) + (g - s) * 128 + (ipg & 127)
            idxh[:, g * 256 : (g + 1) * 256] = d.astype(np.int16)

    in_maps = []
    for cidx in range(N_CORES):
        in_maps.append(
            {
                "xt": np.ascontiguousarray(
                    xf[cidx * B_LOC : (cidx + 1) * B_LOC].reshape(GPX, C)
                ),
                "wt": wt,
                "idxh": idxh,
                "idxs": idxs,
            }
        )
    return in_maps


def kernel(x, w, perm):
    global LAST_RESULT
    _install_ntff_shim()
    from concourse.bass_utils import run_bass_kernel_spmd

    x = np.asarray(x, dtype=np.float32)
    w = np.asarray(w, dtype=np.float32)
    perm = np.asarray(perm)

    if "nc" not in _STATE:
        _STATE["nc"] = _build_kernel()
    nc = _STATE["nc"]

    in_maps = _host_prep(x, w, perm)
    res = run_bass_kernel_spmd(nc, in_maps, core_ids=list(range(N_CORES)))
    LAST_RESULT = res
    out = np.concatenate(
        [r["out"].reshape(B_LOC, C, H, W) for r in res.results], axis=0
    )
    return out.astype(np.float32)


# revision 13
# speedup vs baseline: 1.1672x; 1.1326x over previous
"""Trainium2 Bass kernel: per-pixel channel shuffle + 3x3 conv (stride 1, pad 1).

Problem: x [32,256,56,56] f32, w [256,256,3,3] f32 (OIHW), perm [3136,256] i32;
out[b,:,h,w] = conv3x3(xs)[b,:,h,w] where xs[b,:,l] = x[b, perm[l,:], l].

Strategy (8 NeuronCores, data-parallel over batch, 4 batches/core):
  The 4 local batches form one continuous 12544-pixel stream (98 groups of
  128 px).  Per scatter call (k<=7 groups): DMA a [128, k*256] bf16 tile
  (partition = pixel-within-group, free = group-major channels), one big
  GPSIMD local_scatter applies each pixel's inverse channel permutation AND
  rearranges channels ct-major (dst index table built on host, period 49
  groups = 2 images), then one batched DMA-transpose per 128-channel ic-tile
  ([128, k*128] -> [128, k, 128]) lands [channel, pixel-run] data in a
  staging ring.  DVE copies insert 57-wide row padding (56 px + 1 zero) into
  per-batch image planes [128, 58*57] (+1 lead zero col) so the 3x3 conv is
  pure implicit GEMM: per batch 7 row-groups x 2 oc-tiles x 18 matmuls
  (9 taps x 2 ic-tiles) of [128 x 456] accumulate in PSUM; ACT evicts,
  scalar-queue DMA strips the padding on the way to HBM.  PE runs only conv
  matmuls (~192us roofline/core); GPSIMD ~125us; all overlapped.
"""

import sys
import types
import numpy as np

_STATE = {}
LAST_RESULT = None

B, C, H, W = 32, 256, 56, 56
HW = H * W          # 3136
N_CORES = 8
B_LOC = B // N_CORES
GPX = B_LOC * HW    # 12544 pixels per core
NGRP = GPX // 128   # 98 groups of 128 px
PERIOD = 49         # idx table period in groups (2 images: 49*128 = 2*3136)
ROWW = 57           # padded row width (56 + 1 zero col)
REG = 1 + 58 * ROWW + 1   # per-ic-tile plane region (lead zero + 58 rows + slack)
NG = 8 * ROWW       # conv group free size = 456 (8 output rows)

# scatter calls: (start_group, n_groups). Small head calls start the conv early.
CALLS = [(0, 2), (2, 2), (4, 3)] + [(7 + 7 * i, 7) for i in range(13)]
STG_SLOTS = 6


def _install_ntff_shim():
    # antenv.axon_hooks is absent in some images; provide it so trace=True
    # (BASS_TRACE=1) can capture NTFF profiles instead of crashing.
    name = "antenv.axon_hooks"
    if name in sys.modules:
        return
    try:
        import antenv  # noqa: F401

        m = types.ModuleType(name)
        m._hook = None
        m.set_axon_ntff_profile_hook = lambda h: setattr(m, "_hook", h)
        m.get_axon_ntff_profile_hook = lambda: m._hook
        sys.modules[name] = m
        setattr(sys.modules["antenv"], "axon_hooks", m)
        from trn_agent_boot.trn_boot import _ntff_profile_via_ctypes

        hook = _ntff_profile_via_ctypes("/opt/axon/libaxon_pjrt.so")
        if hook is not None:
            m.set_axon_ntff_profile_hook(hook)
    except Exception:
        pass


def _copy_segments():
    """Per call: list of (ct, slot_off, plane_idx, plane_off, kind, n)
    kind 'rows': full rows block (src [128, n*56] -> dst [[57,n],[1,56]])
    kind 'part': partial row of n px."""
    segs = []
    ready = []  # per call: list of (b, g) conv groups newly ready
    done_rows = 0
    issued = set()
    for m, (s, k) in enumerate(CALLS):
        p0, p1 = 128 * s, 128 * (s + k)
        cur = []
        a = p0
        while a < p1:
            r = a // 56
            row_end = 56 * (r + 1)
            bnd = min(p1, row_end)
            b_idx = r // 56
            rl = r % 56
            if a == 56 * r and bnd == row_end:
                # run of full rows: extend while same batch
                r2 = r
                while (
                    56 * (r2 + 1) <= p1
                    and (r2 // 56) == b_idx
                ):
                    r2 += 1
                n = r2 - r
                cur.append((a - p0, b_idx, 1 + (1 + rl) * ROWW, "rows", n))
                a = 56 * r2
            else:
                n = bnd - a
                cur.append((a - p0, b_idx, 1 + (1 + rl) * ROWW + (a - 56 * r), "part", n))
                a = bnd
        segs.append(cur)
        done_rows = p1 // 56
        newly = []
        for b in range(B_LOC):
            for g in range(7):
                if (b, g) in issued:
                    continue
                if 56 * b + min(8 * g + 9, 56) <= done_rows:
                    issued.add((b, g))
                    newly.append((b, g))
        ready.append(newly)
    assert len(issued) == B_LOC * 7
    return segs, ready


def _build_kernel():
    import concourse.bass as bass
    import concourse.mybir as mybir
    from concourse import bacc, tile
    from contextlib import ExitStack

    F32 = mybir.dt.float32
    BF16 = mybir.dt.bfloat16
    I16 = mybir.dt.int16

    nc = bacc.Bacc("TRN2", target_bir_lowering=False, debug=False, num_devices=N_CORES)

    xt = nc.dram_tensor("xt", [GPX, C], BF16, kind="ExternalInput")
    wt = nc.dram_tensor("wt", [36, 128, 128], BF16, kind="ExternalInput")
    idxh = nc.dram_tensor("idxh", [128, 7 * 256], I16, kind="ExternalInput")
    idxs = nc.dram_tensor("idxs", [128, PERIOD * 256], I16, kind="ExternalInput")
    out = nc.dram_tensor("out", [B_LOC, C, HW], BF16, kind="ExternalOutput")

    segs, ready = _copy_segments()

    with tile.TileContext(nc) as tc, ExitStack() as ctx:
        const = ctx.enter_context(tc.tile_pool(name="const", bufs=1))

        # Pre-warm the GPSIMD local_scatter library (~6us IRAM load).
        dd = const.tile([16, 256], BF16, name="dd", tag="dd")
        nc.vector.memset(dd[:, :], 0.0)
        didx = const.tile([16, 16], I16, name="didx", tag="didx")
        nc.vector.memset(didx[:, :], -1)
        dout = const.tile([16, 256], BF16, name="dout", tag="dout")
        nc.gpsimd.local_scatter(
            out_ap=dout[:, :], data_ap=dd[:, :], idxs_ap=didx[:, :],
            channels=16, num_elems=256, num_idxs=16,
        )

        idxh_sb = const.tile([128, 7 * 256], I16)
        idxs_sb = const.tile([128, PERIOD * 256], I16)
        wsb = const.tile([128, 36 * 128], BF16)

        # double-buffered padded image planes, 2 ic-tile regions each
        planes = []
        for pi in range(2):
            pl = const.tile([128, 2 * REG], BF16, name=f"pl{pi}", tag=f"pl{pi}")
            nc.vector.memset(pl[:, :], 0.0)
            planes.append(pl)

        # staging rings for transposed [channel, pixel] data, one per ic-tile
        stg = [
            const.tile([128, STG_SLOTS * 896], BF16, name=f"stg{ct}", tag=f"stg{ct}")
            for ct in range(2)
        ]

        nc.sync.dma_start(out=idxh_sb[:, :], in_=idxh[:, :])
        nc.scalar.dma_start(
            out=wsb[:, :],
            in_=bass.AP(wt, 0, [[128, 128], [128 * 128, 36], [1, 128]]),
        )
        nc.scalar.dma_start(
            out=idxs_sb[:, : 25 * 256], in_=idxs[:, : 25 * 256]
        )
        nc.scalar.dma_start(
            out=idxs_sb[:, 25 * 256 :], in_=idxs[:, 25 * 256 :]
        )

        xin_pool = ctx.enter_context(tc.tile_pool(name="xin", bufs=3))
        sout_pool = ctx.enter_context(tc.tile_pool(name="sout", bufs=3))
        ost_pool = ctx.enter_context(tc.tile_pool(name="ost", bufs=56))
        mpsum_pool = ctx.enter_context(tc.tile_pool(name="mpsum", bufs=6, space="PSUM"))

        # Outputs stage in SBUF as bf16 and ALL out-DMAs issue after the
        # loop: every DMA is framework-serialized around DMA-transposes, so
        # a PE-paced out-DMA ahead of a transpose would throttle the whole
        # supply chain to PE pace.  Deferring them keeps transposes
        # scatter-paced; the outs drain in parallel with PE's backlog.
        pending_outs = []

        def conv_group(b, g):
            pl = planes[b % 2]
            for oct in range(2):
                mp = mpsum_pool.tile([128, NG], F32, name="mp", tag="mp")
                for i in range(18):
                    ct, tap = divmod(i, 9)
                    dh, dw = divmod(tap, 3)
                    q0 = ct * REG + (8 * g + dh) * ROWW + dw
                    widx = (ct * 9 + tap) * 2 + oct
                    nc.tensor.matmul(
                        mp[:, :],
                        lhsT=wsb[:, widx * 128 : (widx + 1) * 128],
                        rhs=pl[:, q0 : q0 + NG],
                        start=(i == 0),
                        stop=(i == 17),
                    )
                ost = ost_pool.tile([128, NG], BF16, name="ost", tag="ost")
                nc.scalar.copy(ost[:, :], mp[:, :])
                pending_outs.append((b, g, oct, ost))

        # xin prefetch 2 calls ahead so transposes (which wait on the
        # scatter) never block the next input load on the sync FIFO
        xins = {}

        def load_xin(m):
            if m >= len(CALLS):
                return
            s, k = CALLS[m]
            kk = k * 256
            xin = xin_pool.tile([128, 7 * 256], BF16, name="xin", tag="xin")
            nc.sync.dma_start(
                out=xin[:, :kk],
                in_=bass.AP(xt, 128 * s * C, [[C, 128], [128 * C, k], [1, C]]),
            )
            xins[m] = xin

        def do_copies(mi):
            slot_i = (mi % STG_SLOTS) * 896
            for src_off, b_idx, pl_off, kind, n in segs[mi]:
                pl = planes[b_idx % 2]
                for ct in range(2):
                    so = slot_i + src_off
                    po = ct * REG + pl_off
                    if kind == "rows":
                        src = stg[ct][:, so : so + 56 * n].rearrange(
                            "p (r x) -> p r x", r=n
                        )
                        dst = pl[:, po : po + n * ROWW].rearrange(
                            "p (r x) -> p r x", r=n
                        )[:, :, 0:56]
                        nc.scalar.copy(dst, src)
                    else:
                        nc.scalar.copy(
                            pl[:, po : po + n], stg[ct][:, so : so + n]
                        )

        load_xin(0)
        load_xin(1)
        for m, (s, k) in enumerate(CALLS):
            kk = k * 256
            load_xin(m + 2)
            xin = xins.pop(m)
            if m < 3:
                iap = idxh_sb[:, s * 256 : (s + k) * 256]
            else:
                sp = (s % PERIOD) * 256
                iap = idxs_sb[:, sp : sp + kk]
            sout = sout_pool.tile([128, 7 * 256], BF16, name="sout", tag="sout")
            nc.gpsimd.local_scatter(
                out_ap=sout[:, :kk],
                data_ap=xin[:, :kk],
                idxs_ap=iap,
                channels=128,
                num_elems=kk,
                num_idxs=kk,
            )
            slot = (m % STG_SLOTS) * 896
            for ct in range(2):
                dst = stg[ct][:, slot : slot + k * 128].rearrange(
                    "p (e l) -> p e l", e=k
                )
                nc.sync.dma_start_transpose(
                    dst, sout[:, ct * k * 128 : (ct + 1) * k * 128]
                )
            # pad-copies on ACT (own SBUF port; DVE contends with GPSIMD).
            # Copies lag one call (their transpose is already done -> the
            # scalar FIFO head never waits on an in-flight scatter) and conv
            # groups lag two calls (so an eviction at the FIFO head only
            # delays copies that PE won't need for another full call).
            if m >= 1:
                do_copies(m - 1)
            if m >= 2:
                for (b, g) in ready[m - 2]:
                    conv_group(b, g)
        do_copies(len(CALLS) - 1)
        for mm in (len(CALLS) - 2, len(CALLS) - 1):
            for (b, g) in ready[mm]:
                conv_group(b, g)
        for b, g, oct, ost in pending_outs:
            nc.sync.dma_start(
                out=out[b, oct * 128 : (oct + 1) * 128, 448 * g : 448 * g + 448],
                in_=ost[:, :].rearrange("p (r x) -> p r x", r=8)[:, :, 0:56],
            )

    nc.compile()
    return nc


def _host_prep(x, w, perm):
    import ml_dtypes

    # pixel-major bf16: [B, HW, C]
    xf = np.ascontiguousarray(
        x.reshape(B, C, HW).transpose(0, 2, 1)
    ).astype(ml_dtypes.bfloat16)

    wt = np.empty((36, 128, 128), dtype=ml_dtypes.bfloat16)
    wf = np.asarray(w, dtype=np.float32)
    for ct in range(2):
        for tap in range(9):
            kh, kw = divmod(tap, 3)
            for oct in range(2):
                i = (ct * 9 + tap) * 2 + oct
                wt[i] = wf[
                    oct * 128 : (oct + 1) * 128, ct * 128 : (ct + 1) * 128, kh, kw
                ].T.astype(ml_dtypes.bfloat16)

    iperm = np.empty((HW, C), dtype=np.int16)
    np.put_along_axis(
        iperm, perm.astype(np.int64), np.arange(C, dtype=np.int16)[None, :], axis=1
    )

    # steady idx table [128, PERIOD*256]: group g, partition p -> pixel
    # (128g+p) % HW; dst = ct_major(k=7): (ip>>7)*896 + (g%7)*128 + (ip&127)
    gg = np.arange(PERIOD)
    pp = np.arange(128)
    l = (128 * gg[:, None] + pp[None, :]) % HW          # [49, 128]
    ip = iperm[l].astype(np.int32)                      # [49, 128, 256]
    j = (gg % 7).astype(np.int32)[:, None, None]
    dst = (ip >> 7) * 896 + j * 128 + (ip & 127)
    idxs = np.ascontiguousarray(
        dst.astype(np.int16).transpose(1, 0, 2).reshape(128, PERIOD * 256)
    )

    # head table for the first 7 groups with call sizes (0,2),(2,2),(4,3)
    idxh = np.empty((128, 7 * 256), dtype=np.int16)
    for (s, k) in CALLS[:3]:
        for g in range(s, s + k):
            lg = (128 * g + pp) % HW
            ipg = iperm[lg].astype(np.int32)            # [128, 256]
            d = (ipg >> 7) * (k * 128) + (g - s) * 128 + (ipg & 127)
            idxh[:, g * 256 : (g + 1) * 256] = d.astype(np.int16)

    in_maps = []
    for cidx in range(N_CORES):
        in_maps.append(
            {
                "xt": np.ascontiguousarray(
                    xf[cidx * B_LOC : (cidx + 1) * B_LOC].reshape(GPX, C)
                ),
                "wt": wt,
                "idxh": idxh,
                "idxs": idxs,
            }
        )
    return in_maps


def kernel(x, w, perm):
    global LAST_RESULT
    _install_ntff_shim()
    from concourse.bass_utils import run_bass_kernel_spmd

    x = np.asarray(x, dtype=np.float32)
    w = np.asarray(w, dtype=np.float32)
    perm = np.asarray(perm)

    if "nc" not in _STATE:
        _STATE["nc"] = _build_kernel()
    nc = _STATE["nc"]

    in_maps = _host_prep(x, w, perm)
    res = run_bass_kernel_spmd(nc, in_maps, core_ids=list(range(N_CORES)))
    LAST_RESULT = res
    out = np.concatenate(
        [r["out"].reshape(B_LOC, C, H, W) for r in res.results], axis=0
    )
    return out.astype(np.float32)
